# revision 1
# baseline (speedup 1.0000x reference)
"""Trainium2 Bass kernel for nn_BSN_76218489635087 (segment_reduce).

Computation (reference):
    h = relu-MLP(x[0])            # [2048, 64]
    s = h @ tr_bags               # [2048, 100000]
    col_max = max over rows       # [100000]
    ref_max = segment_max(col_max, tr_mask, 100)
    y_prob = sigmoid(ref_max @ W4 + b4); y_hat = y_prob >= 0.5

Sharding: tr_bags columns (T) split across 8 cores (12544 padded cols each).
Each core computes the full (replicated) MLP producing hT = h.T [64, 2048]
(as 4 n-chunk tiles so score matmuls start per chunk). All matmul operands
are fp16 (1 cycle/row on the PE, fp32 PSUM accumulation; plain fp32 lowers
to 2 half-speed passes and float32r measured ~2.5x slower than fp16 on HW).

PSUM drain (the bottleneck - every score element must leave PSUM through
ScalarE or VectorE at 1 elem/lane/cycle): every score tile [128, 2048] is
ScalarE-copied to fp16 SBUF, then VectorE folds it with a max-tree
(tensor_max halvings at the 2x packed fp16 rate, then reduce_max).
All-ACT measured fastest: mixing in VectorE-direct tiles exposes the
~1.9us PE refill latency as a ScalarE stall of the same size, so the
loop is paced at ~2.0us/tile by the ScalarE copy stream through the
2-deep (4-bank) PSUM pipeline either way, and all-ACT frees VectorE.

Host gathers the 100352 column maxes, does the segment-max + final
100->1 dot + sigmoid.
"""

import sys
import os

for _p in ("/opt/trn_rl_repo", "/root/.axon_site/_ro/pypackages", "/root/.axon_site"):
    if _p not in sys.path and os.path.isdir(_p):
        sys.path.append(_p)

import numpy as np

from concourse import bass, bacc, tile, mybir
from concourse.bass_utils import run_bass_kernel_spmd

# Problem constants (hardcoded per harness contract)
N = 2048          # instances
D = 512           # input features
T = 100000        # reference instance columns
R = 100           # num references (segments)
NCORES = 8
TPC = 12544       # padded columns per core (= 98 * 128); 8*12544 = 100352
NT = TPC // 128   # 98 column-tiles per core

F32 = mybir.dt.float32
F32R = mybir.dt.float32r
F16 = mybir.dt.float16

# Of every 8 score tiles, this many drain via the ACT-copy path (rest DVE).
ACT_TILES = frozenset({0, 1, 2, 4, 6})

USE_ALT = os.environ.get("K_ALT", "0") == "1"      # row-group alternation
USE_SPLIT = os.environ.get("K_SPLIT", "1") == "1"  # ACT/DVE drain split
PDIRECT = int(os.environ.get("K_PDIRECT", "98"))   # ACT-path tiles per 98 (rest DVE-direct)
TAILENG = os.environ.get("K_TAILENG", "gpsimd")    # gpsimd | vector


def _mm(nc, out, lhsT, rhs, **kw):
    nc.tensor.matmul(out, lhsT, rhs, **kw)


def _build_program():
    nc = bacc.Bacc("TRN2", target_bir_lowering=False, debug=False, num_devices=NCORES)

    xT_d = nc.dram_tensor("xT", [D, N], F16, kind="ExternalInput")
    w1_d = nc.dram_tensor("w1", [D, 256], F16, kind="ExternalInput")
    w2_d = nc.dram_tensor("w2", [256, 128], F16, kind="ExternalInput")
    w3_d = nc.dram_tensor("w3", [128, 64], F16, kind="ExternalInput")
    b1_d = nc.dram_tensor("b1", [256, 1], F32, kind="ExternalInput")
    b2_d = nc.dram_tensor("b2", [128, 1], F32, kind="ExternalInput")
    b3_d = nc.dram_tensor("b3", [64, 1], F32, kind="ExternalInput")
    bags_shape = [128, TPC // 2] if USE_ALT else [64, TPC]
    bags_d = nc.dram_tensor("bags", bags_shape, F16, kind="ExternalInput")
    out_d = nc.dram_tensor("colmax_out", [128, NT], F32, kind="ExternalOutput")

    relu = mybir.ActivationFunctionType.Relu
    copyf = mybir.ActivationFunctionType.Copy
    amax = mybir.AluOpType.max

    with tile.TileContext(nc) as tc:
        with (
            tc.tile_pool(name="const", bufs=1) as cpool,
            tc.tile_pool(name="scr", bufs=4) as spool,
            tc.tile_pool(name="psum", bufs=2, space="PSUM") as ppool,
        ):
            # ---- load everything ----
            xT_sb = []
            for k in range(4):
                t = cpool.tile([128, N], F16, tag=f"xT{k}", name=f"xT{k}")
                nc.sync.dma_start(t[:], xT_d[128 * k : 128 * (k + 1), :])
                xT_sb.append(t)
            w1_sb = []
            for k in range(4):
                t = cpool.tile([128, 256], F16, tag=f"w1{k}", name=f"w1s{k}")
                nc.sync.dma_start(t[:], w1_d[128 * k : 128 * (k + 1), :])
                w1_sb.append(t)
            w2_sb = []
            for k in range(2):
                t = cpool.tile([128, 128], F16, tag=f"w2{k}", name=f"w2s{k}")
                nc.sync.dma_start(t[:], w2_d[128 * k : 128 * (k + 1), :])
                w2_sb.append(t)
            w3_sb = cpool.tile([128, 64], F16, tag="w3")
            nc.sync.dma_start(w3_sb[:], w3_d[:, :])
            b1_sb = []
            for m in range(2):
                t = cpool.tile([128, 1], F32, tag=f"b1{m}", name=f"b1s{m}")
                nc.sync.dma_start(t[:], b1_d[128 * m : 128 * (m + 1), :])
                b1_sb.append(t)
            b2_sb = cpool.tile([128, 1], F32, tag="b2")
            nc.sync.dma_start(b2_sb[:], b2_d[:, :])
            b3_sb = cpool.tile([64, 1], F32, tag="b3")
            nc.sync.dma_start(b3_sb[:], b3_d[:, :])

            bags_sb = cpool.tile(bags_shape, F16, tag="bags")
            nc.sync.dma_start(bags_sb[:], bags_d[:, :])

            g1_sb = [
                cpool.tile([128, N], F16, tag=f"g1{m}", name=f"g1s{m}")
                for m in range(2)
            ]
            g2_sb = cpool.tile([128, N], F16, tag="g2")
            hT_sb = [
                cpool.tile([64, 512], F16, tag=f"hT{j}", name=f"hT{j}")
                for j in range(4)
            ]
            colmax_sb = cpool.tile([128, NT], F32, tag="colmax")

            # ---- layer 1: g1 = relu(W1.T @ xT + b1) -> [256, 2048] as 2 tiles
            for m in range(2):
                ps = ppool.tile([128, N], F32, tag="ps", name=f"psl1{m}")
                for j in range(4):
                    for k in range(4):
                        _mm(
                            nc,
                            ps[:, 512 * j : 512 * (j + 1)],
                            w1_sb[k][:, 128 * m : 128 * (m + 1)],
                            xT_sb[k][:, 512 * j : 512 * (j + 1)],
                            start=(k == 0),
                            stop=(k == 3),
                        )
                if m == 0:
                    nc.scalar.activation(
                        g1_sb[m][:, :], ps[:, :], relu, bias=b1_sb[m][:, :]
                    )
                else:
                    # relu(x + b) on VectorE so both L1 relus run concurrently
                    nc.vector.tensor_scalar(
                        out=g1_sb[m][:, :], in0=ps[:, :],
                        scalar1=b1_sb[m][:, :], scalar2=0.0,
                        op0=amax if False else mybir.AluOpType.add, op1=amax,
                    )

            # ---- layer 2: g2 = relu(W2.T @ g1 + b2) -> [128, 2048]
            ps = ppool.tile([128, N], F32, tag="ps", name="psl2")
            for j in range(4):
                for k in range(2):
                    _mm(
                        nc,
                        ps[:, 512 * j : 512 * (j + 1)],
                        w2_sb[k][:, :],
                        g1_sb[k][:, 512 * j : 512 * (j + 1)],
                        start=(k == 0),
                        stop=(k == 1),
                    )
            nc.scalar.activation(
                g2_sb[:, 0:1024], ps[:, 0:1024], relu, bias=b2_sb[:, :]
            )
            nc.vector.tensor_scalar(
                out=g2_sb[:, 1024:2048], in0=ps[:, 1024:2048],
                scalar1=b2_sb[:, :], scalar2=0.0,
                op0=mybir.AluOpType.add, op1=amax,
            )

            # ---- layer 3: hT = relu(W3.T @ g2 + b3) -> [64, 2048], then
            #      duplicated into partitions 64:128 for row-group alternation
            ps = ppool.tile([128, N], F32, tag="ps", name="psl3")
            for j in range(4):
                _mm(
                    nc,
                    ps[0:64, 512 * j : 512 * (j + 1)],
                    w3_sb[:, :],
                    g2_sb[:, 512 * j : 512 * (j + 1)],
                    start=True,
                    stop=True,
                )
            for j in range(4):
                nc.scalar.activation(
                    hT_sb[j][:, :], ps[0:64, 512 * j : 512 * (j + 1)], relu,
                    bias=b3_sb[:, :],
                )

            # ---- scores: tile i lives in partition half i%2, col block i//2
            # Tails are emitted one tile late so a direct tile's PSUM reduce
            # sits BEFORE the previous tail in DVE program order - its PSUM
            # slot then frees ~2us earlier and ScalarE never stalls on refill.
            deferred = None

            def emit_tail(scr, i):
                t1 = spool.tile([128, N // 2], F16, tag="t1", name=f"t1_{i}")
                nc.vector.tensor_max(t1[:, :], scr[:, 0 : N // 2], scr[:, N // 2 : N])
                t2 = spool.tile([128, N // 4], F16, tag="t2", name=f"t2_{i}")
                nc.vector.tensor_max(t2[:, :], t1[:, 0 : N // 4], t1[:, N // 4 : N // 2])
                nc.vector.reduce_max(
                    colmax_sb[:, i : i + 1], t2[:, :], axis=mybir.AxisListType.X
                )

            for i in range(NT):
                lhsT = bags_sb[:, 128 * i : 128 * (i + 1)]
                ps = ppool.tile([128, N], F32, tag="ps", name=f"pss{i}")
                for j in range(4):
                    _mm(
                        nc,
                        ps[:, 512 * j : 512 * (j + 1)],
                        lhsT,
                        hT_sb[j][:, :],
                        start=True,
                        stop=True,
                    )
                if USE_SPLIT and (i % 98) < PDIRECT:
                    scr = spool.tile([128, N], F16, tag="scr", name=f"scr{i}")
                    nc.scalar.activation(scr[:, :], ps[:, :], copyf)
                    if deferred is not None:
                        emit_tail(*deferred)
                    deferred = (scr, i)
                else:
                    nc.vector.reduce_max(
                        colmax_sb[:, i : i + 1], ps[:, :], axis=mybir.AxisListType.X
                    )
                    if deferred is not None:
                        emit_tail(*deferred)
                        deferred = None
            if deferred is not None:
                emit_tail(*deferred)

            nc.sync.dma_start(out_d[:, :], colmax_sb[:])

    nc.compile()
    return nc


_CACHED = {}


def _get_program():
    if "nc" not in _CACHED:
        _CACHED["nc"] = _build_program()
    return _CACHED["nc"]


def _run_device(in_maps, trace=False):
    nc = _get_program()
    try:
        return run_bass_kernel_spmd(nc, in_maps, list(range(NCORES)), trace=trace)
    except ModuleNotFoundError:
        if not trace:
            raise
        return run_bass_kernel_spmd(nc, in_maps, list(range(NCORES)), trace=False)


def _prep_inputs(x, tr_bags, W1, b1, W2, b2, W3, b3):
    xT = np.ascontiguousarray(np.asarray(x, np.float32)[0].T)  # [512, 2048]
    bags = np.asarray(tr_bags, np.float32)
    bags_pad = np.zeros((64, NCORES * TPC), np.float32)
    bags_pad[:, :T] = bags
    base = {
        "xT": xT.astype(np.float16),
        "w1": np.ascontiguousarray(np.asarray(W1, np.float32).astype(np.float16)),
        "w2": np.ascontiguousarray(np.asarray(W2, np.float32).astype(np.float16)),
        "w3": np.ascontiguousarray(np.asarray(W3, np.float32).astype(np.float16)),
        "b1": np.asarray(b1, np.float32).reshape(256, 1).copy(),
        "b2": np.asarray(b2, np.float32).reshape(128, 1).copy(),
        "b3": np.asarray(b3, np.float32).reshape(64, 1).copy(),
    }
    in_maps = []
    for c in range(NCORES):
        shard = bags_pad[:, c * TPC : (c + 1) * TPC]
        if USE_ALT:
            sh = shard.reshape(64, NT, 128)
            packed = np.empty((128, TPC // 2), np.float32)
            # even tiles -> partitions 0:64, odd tiles -> 64:128, col block i//2
            packed[0:64] = sh[:, 0::2, :].reshape(64, -1)
            packed[64:128] = sh[:, 1::2, :].reshape(64, -1)
        else:
            packed = shard
        m = dict(base)
        m["bags"] = np.ascontiguousarray(packed.astype(np.float16))
        in_maps.append(m)
    return in_maps


def _finish_host(colmax, tr_mask, W4, b4):
    tm = np.asarray(tr_mask)
    boundaries = np.searchsorted(tm, np.arange(R + 1))
    ref_max = np.full(R, -np.inf, np.float32)
    nonempty = boundaries[1:] > boundaries[:-1]
    if nonempty.any():
        starts = boundaries[:-1][nonempty]
        ref_max[nonempty] = np.maximum.reduceat(colmax, starts)[: nonempty.sum()]
    z = ref_max.astype(np.float32) @ np.asarray(W4, np.float32) + np.asarray(
        b4, np.float32
    )
    y_prob = (1.0 / (1.0 + np.exp(-z.astype(np.float64)))).astype(np.float32).squeeze()
    y_hat = np.float32(1.0) if y_prob >= 0.5 else np.float32(0.0)
    return np.asarray(y_prob, np.float32), np.asarray(y_hat, np.float32)


def kernel(x, tr_bags, tr_mask, W1, b1, W2, b2, W3, b3, W4, b4, _trace=False):
    in_maps = _prep_inputs(x, tr_bags, W1, b1, W2, b2, W3, b3)
    res = _run_device(in_maps, trace=_trace)
    colmax_parts = []
    for c in range(NCORES):
        cm = res.results[c]["colmax_out"]  # [128, NT]
        colmax_parts.append(np.asarray(cm).T.reshape(-1))  # [TPC], col-major by tile
    colmax = np.concatenate(colmax_parts)[:T]
    out = _finish_host(colmax, tr_mask, W4, b4)
    if _trace:
        return out, res
    return out



# revision 2
# speedup vs baseline: 1.0150x; 1.0150x over previous
"""Trainium2 Bass kernel v10 for nn_BSN_76218489635087 (segment_reduce).

T columns sharded 8 ways (12544 padded per core).  Per core:

Head: PE warmup matmuls on zeros during the DMA wait (HAM ramps to 2.4GHz
only under sustained full-128-partition matmul activity); DMA order
w1 -> xT (column-chunked) -> small weights -> bags (16 chunks); MLP
pipelined in 4 N-chunks of 512 producing hT chunks [128, 512] fp16
(rows 64:128 zeroed: K=128 engagement keeps the PE un-throttled).

Score tile i (128 T-cols x 2048 N):
  PE:  psB <- chunks 2,3 (+1 zero-filler pass), psA <- chunks 0,1
       (psA/psB are separate [128,1024] PSUM pool tiles so each half
       frees as soon as its reader is done)
  ACT: two 512-col copies psB -> scrA fp16 (starts right after chunk 2)
  DVE: one custom MAXTT_REDUCE_ANT: streams in0=psA (fp32 PSUM) +
       in1=scrA (fp16 SBUF), elementwise max, accum-max over the free
       dim -> colmax[:, i].  No tail reductions anywhere.

Host: segment-max over gathered col maxes + final dot + sigmoid.
"""

import sys
import os

for _p in ("/opt/trn_rl_repo", "/root/.axon_site/_ro/pypackages", "/root/.axon_site"):
    if _p not in sys.path and os.path.isdir(_p):
        sys.path.append(_p)

import numpy as np

from concourse import bass, bacc, tile, mybir
from concourse.bass_utils import run_bass_kernel_spmd

# ---- register the custom DVE op (documented extension point) --------------
from concourse import dve_ops as _dvo
from concourse.dve_spec import Spec as _Spec, Src0 as _Src0, Src1 as _Src1, maxx as _maxx

if "MAXTT_REDUCE_ANT" not in _dvo._SUB_OPCODE_FOR_NAME:
    _MAXTT = _dvo.DveOp(
        "MAXTT_REDUCE_ANT",
        _Spec(body=_maxx(_Src0, _Src1), accum=_maxx),
        subdim=False,
        uops_sha={"v3": "e8861e626b8ad62a", "v4": "7f8046c2b2ccaaf7"},
    )
    _dvo.OPS.append(_MAXTT)
    _dvo.CUSTOM_DVE_SPECS[_MAXTT.name] = _MAXTT.spec
    _dvo._SUB_OPCODE_FOR_NAME[_MAXTT.name] = max(_dvo._SUB_OPCODE_FOR_NAME.values()) + 1
else:
    _MAXTT = next(op for op in _dvo.OPS if op.name == "MAXTT_REDUCE_ANT")

N = 2048
D = 512
T = 100000
R = 100
NCORES = 8
TPC = 12544
NT = TPC // 128  # 98

F32 = mybir.dt.float32
F16 = mybir.dt.float16

KFILL = int(os.environ.get("K_FILL", "1"))      # zero filler passes per tile
NWARM = int(os.environ.get("K_WARM", "4"))     # PE warmup matmuls on zeros


def _build_program():
    nc = bacc.Bacc("TRN2", target_bir_lowering=False, debug=False, num_devices=NCORES)

    xT_d = nc.dram_tensor("xT", [128, 16, 512], F16, kind="ExternalInput")
    w1_d = nc.dram_tensor("w1", [128, 4, 256], F16, kind="ExternalInput")
    w2_d = nc.dram_tensor("w2", [128, 2, 128], F16, kind="ExternalInput")
    w3_d = nc.dram_tensor("w3", [128, 64], F16, kind="ExternalInput")
    bcat_d = nc.dram_tensor("bcat", [128, 4], F32, kind="ExternalInput")
    bags_d = nc.dram_tensor("bags", [64, TPC], F16, kind="ExternalInput")
    out_d = nc.dram_tensor("colmax_out", [128, NT], F32, kind="ExternalOutput")

    relu = mybir.ActivationFunctionType.Relu
    copyf = mybir.ActivationFunctionType.Copy
    amax = mybir.AluOpType.max
    aadd = mybir.AluOpType.add

    with tile.TileContext(nc) as tc:
        with (
            tc.tile_pool(name="const", bufs=1) as cpool,
            tc.tile_pool(name="psA", bufs=2, space="PSUM") as apool,
            tc.tile_pool(name="psB", bufs=2, space="PSUM") as bpool,
        ):
            # ---- zero tiles (memset first: no deps) ----
            zbags_sb = cpool.tile([128, 128], F16, tag="zbags")
            nc.vector.memset(zbags_sb[:, :], 0.0)
            zrhs_sb = cpool.tile([128, 512], F16, tag="zrhs")
            nc.vector.memset(zrhs_sb[:, :], 0.0)
            hT_sb = [
                cpool.tile([128, 512], F16, tag=f"hT{j}", name=f"hT{j}")
                for j in range(4)
            ]
            for j in range(4):
                nc.vector.memset(hT_sb[j][64:128, :], 0.0)

            # ---- DMA loads (multi-queue) ----
            # gpsimd queue: small weights first
            w1_sb = cpool.tile([128, 4, 256], F16, tag="w1p")
            nc.gpsimd.dma_start(w1_sb[:, :, :], w1_d[:, :, :])
            bcat_sb = cpool.tile([128, 4], F32, tag="bcat")
            nc.gpsimd.dma_start(bcat_sb[:, :], bcat_d[:, :])
            b1_sb = [bcat_sb[:, 0:1], bcat_sb[:, 1:2]]
            b2_sb = bcat_sb[:, 2:3]
            b3_sb = bcat_sb[0:64, 3:4]
            w2p_sb = cpool.tile([128, 2, 128], F16, tag="w2p")
            nc.gpsimd.dma_start(w2p_sb[:, :, :], w2_d[:, :, :])
            w2_sb = [w2p_sb[:, 0, :], w2p_sb[:, 1, :]]
            w3_sb = cpool.tile([128, 64], F16, tag="w3")
            nc.gpsimd.dma_start(w3_sb[:], w3_d[:, :])
            # sync queue: xT as 4 strided column-chunk transfers; chunk c
            # covers cols 512c:512(c+1) of ALL k-slices, so L1 chunk j
            # unblocks after one transfer
            # chunk-major xT: [128, 4c+k, 512]; each chunk DMA contiguous
            xT_sb = cpool.tile([128, 16, 512], F16, tag="xTp")
            for c in (2, 3, 0, 1):
                nc.sync.dma_start(
                    xT_sb[:, 4 * c : 4 * (c + 1), :],
                    xT_d[:, 4 * c : 4 * (c + 1), :],
                )
            # bags: real rows on gpsimd queue after the small weights;
            # zero rows 64:128 via idle ACT (memzero) + DVE (memset) early
            bags_sb = cpool.tile([128, TPC], F16, tag="bags")
            nc.scalar.memzero(bags_sb[64:128, 0 : TPC // 4])
            nc.scalar.memzero(bags_sb[64:128, TPC // 4 : TPC // 2])
            nc.vector.memset(bags_sb[64:128, TPC // 2 : 3 * TPC // 4], 0.0)
            nc.vector.memset(bags_sb[64:128, 3 * TPC // 4 : TPC], 0.0)
            BCH = TPC // 8
            for c in range(8):
                nc.gpsimd.dma_start(
                    bags_sb[0:64, BCH * c : BCH * (c + 1)],
                    bags_d[:, BCH * c : BCH * (c + 1)],
                )

            g1_sb = [
                cpool.tile([128, N], F16, tag=f"g1{m}", name=f"g1s{m}")
                for m in range(2)
            ]
            g2_sb = cpool.tile([128, N], F16, tag="g2")
            colmax_sb = cpool.tile([128, NT], F32, tag="colmax")
            scrA = [
                cpool.tile([128, 1024], F16, tag=f"scrA{r}", name=f"scrA{r}")
                for r in range(4)
            ]
            trash = [
                cpool.tile([128, 1024], F16, tag=f"trash{r}", name=f"trash{r}")
                for r in range(1)
            ]

            # ---- PE warmup on zeros (during DMA wait) ----
            for w in range(NWARM):
                pw = apool.tile([128, 1024], F32, tag="psA", name=f"warm{w}")
                nc.tensor.matmul(pw[:, 0:512], zbags_sb[:, :], zrhs_sb[:, :],
                                 start=True, stop=True)

            # ---- MLP, pipelined in 4 N-chunks of 512 ----
            # Chunk order 2,3,0,1: score tile 0 consumes hT2/hT3 first,
            # so it can start after just two MLP chunks.
            for j in (2, 3, 0, 1):
                psa = apool.tile([128, 1024], F32, tag="psA", name=f"psmlpa{j}")
                psb = bpool.tile([128, 1024], F32, tag="psB", name=f"psmlpb{j}")
                sl = slice(512 * j, 512 * (j + 1))
                # L1 -> [256, 512] two m-halves into psa
                for m in range(2):
                    for k in range(4):
                        nc.tensor.matmul(
                            psa[:, 512 * m : 512 * (m + 1)],
                            w1_sb[:, k, 128 * m : 128 * (m + 1)],
                            xT_sb[:, 4 * j + k, :],
                            start=(k == 0),
                            stop=(k == 3),
                        )
                nc.scalar.activation(g1_sb[0][:, sl], psa[:, 0:512], relu,
                                     bias=b1_sb[0])
                nc.vector.tensor_scalar(
                    out=g1_sb[1][:, sl], in0=psa[:, 512:1024],
                    scalar1=b1_sb[1], scalar2=0.0, op0=aadd, op1=amax,
                )
                # L2 -> [128, 512] into psb[:, 0:512]
                for k in range(2):
                    nc.tensor.matmul(
                        psb[:, 0:512], w2_sb[k], g1_sb[k][:, sl],
                        start=(k == 0), stop=(k == 1),
                    )
                nc.vector.tensor_scalar(
                    out=g2_sb[:, sl], in0=psb[:, 0:512],
                    scalar1=b2_sb, scalar2=0.0, op0=aadd, op1=amax,
                )
                # L3 -> [64, 512] into psb[0:64, 512:1024]
                nc.tensor.matmul(
                    psb[0:64, 512:1024], w3_sb[:, :], g2_sb[:, sl],
                    start=True, stop=True,
                )
                nc.scalar.activation(
                    hT_sb[j][0:64, :], psb[0:64, 512:1024], relu, bias=b3_sb
                )

            # ---- score loop ----
            for i in range(NT):
                lhsT = bags_sb[:, 128 * i : 128 * (i + 1)]
                psb = bpool.tile([128, 1024], F32, tag="psB", name=f"pssb{i}")
                psa = apool.tile([128, 1024], F32, tag="psA", name=f"pssa{i}")
                # B half: chunk 2 (clean, so ACT copy 1 starts earliest),
                # then chunk 3 with the zero-filler passes in its group
                nc.tensor.matmul(psb[:, 0:512], lhsT, hT_sb[2][:, :],
                                 start=True, stop=True)
                nc.tensor.matmul(psb[:, 512:1024], lhsT, hT_sb[3][:, :],
                                 start=True, stop=(KFILL == 0))
                for _ in range(KFILL):
                    nc.tensor.matmul(psb[:, 512:1024], zbags_sb[:, :], hT_sb[3][:, :],
                                     start=False, stop=True)
                # A half: chunks 0, 1
                nc.tensor.matmul(psa[:, 0:512], lhsT, hT_sb[0][:, :],
                                 start=True, stop=True)
                nc.tensor.matmul(psa[:, 512:1024], lhsT, hT_sb[1][:, :],
                                 start=True, stop=True)
                # ACT: two 512-col copies so the first starts right after chunk 2
                sA = scrA[i % 4]
                nc.scalar.activation(sA[:, 0:512], psb[:, 0:512], copyf)
                nc.scalar.activation(sA[:, 512:1024], psb[:, 512:1024], copyf)
                # DVE: drain psa + fold scrA, reduce to colmax column
                nc.vector._custom_dve(
                    _MAXTT,
                    out=trash[0][:, :],
                    in0=psa[:, :],
                    in1=sA[:, :],
                    accum_out=colmax_sb[:, i : i + 1],
                )

            nc.sync.dma_start(out_d[:, :], colmax_sb[:])

    nc.compile()
    return nc


_CACHED = {}


def _get_program():
    if "nc" not in _CACHED:
        _CACHED["nc"] = _build_program()
    return _CACHED["nc"]


def _run_device(in_maps, trace=False):
    nc = _get_program()
    try:
        return run_bass_kernel_spmd(nc, in_maps, list(range(NCORES)), trace=trace)
    except ModuleNotFoundError:
        if not trace:
            raise
        return run_bass_kernel_spmd(nc, in_maps, list(range(NCORES)), trace=False)


def _prep_inputs(x, tr_bags, W1, b1, W2, b2, W3, b3):
    xT = np.ascontiguousarray(np.asarray(x, np.float32)[0].T)  # [512, 2048]
    xTp = (xT.astype(np.float16).reshape(4, 128, 4, 512)
           .transpose(1, 2, 0, 3).reshape(128, 16, 512))
    w1p = (np.asarray(W1, np.float32).astype(np.float16)
           .reshape(4, 128, 256).transpose(1, 0, 2))
    w2p = (np.asarray(W2, np.float32).astype(np.float16)
           .reshape(2, 128, 128).transpose(1, 0, 2))
    bcat = np.zeros((128, 4), np.float32)
    bcat[:, 0] = np.asarray(b1, np.float32)[0:128]
    bcat[:, 1] = np.asarray(b1, np.float32)[128:256]
    bcat[:, 2] = np.asarray(b2, np.float32)
    bcat[0:64, 3] = np.asarray(b3, np.float32)
    bags = np.asarray(tr_bags, np.float32)
    bags_pad = np.zeros((64, NCORES * TPC), np.float32)
    bags_pad[:, :T] = bags
    base = {
        "xT": np.ascontiguousarray(xTp),
        "w1": np.ascontiguousarray(w1p),
        "w2": np.ascontiguousarray(w2p),
        "w3": np.ascontiguousarray(np.asarray(W3, np.float32).astype(np.float16)),
        "bcat": bcat,
    }
    in_maps = []
    for c in range(NCORES):
        m = dict(base)
        m["bags"] = np.ascontiguousarray(
            bags_pad[:, c * TPC : (c + 1) * TPC].astype(np.float16)
        )
        in_maps.append(m)
    return in_maps


def _finish_host(colmax, tr_mask, W4, b4):
    tm = np.asarray(tr_mask)
    boundaries = np.searchsorted(tm, np.arange(R + 1))
    ref_max = np.full(R, -np.inf, np.float32)
    nonempty = boundaries[1:] > boundaries[:-1]
    if nonempty.any():
        starts = boundaries[:-1][nonempty]
        ref_max[nonempty] = np.maximum.reduceat(colmax, starts)[: nonempty.sum()]
    z = ref_max.astype(np.float32) @ np.asarray(W4, np.float32) + np.asarray(
        b4, np.float32
    )
    y_prob = (1.0 / (1.0 + np.exp(-z.astype(np.float64)))).astype(np.float32).squeeze()
    y_hat = np.float32(1.0) if y_prob >= 0.5 else np.float32(0.0)
    return np.asarray(y_prob, np.float32), np.asarray(y_hat, np.float32)


def kernel(x, tr_bags, tr_mask, W1, b1, W2, b2, W3, b3, W4, b4, _trace=False):
    in_maps = _prep_inputs(x, tr_bags, W1, b1, W2, b2, W3, b3)
    res = _run_device(in_maps, trace=_trace)
    colmax_parts = []
    for c in range(NCORES):
        cm = res.results[c]["colmax_out"]  # [128, NT]
        colmax_parts.append(np.asarray(cm).T.reshape(-1))
    colmax = np.concatenate(colmax_parts)[:T]
    out = _finish_host(colmax, tr_mask, W4, b4)
    if _trace:
        return out, res
    return out


# revision 3
# speedup vs baseline: 1.0490x; 1.0335x over previous
"""Trainium2 Bass kernel v10 for nn_BSN_76218489635087 (segment_reduce).

T columns sharded 8 ways (12544 padded per core).  Per core:

Head: PE warmup matmuls on zeros during the DMA wait (the PE HAM clock
gate releases 2.4GHz only under sustained full-128-partition matmul
activity; K=64 matmuls stay throttled at 1.2GHz forever).  xT is sent
chunk-major [128, 16, 512] so each 512-column chunk is one contiguous
DMA, issued in the order the MLP consumes them (2,3,0,1); weights/bags
go on the gpsimd queue; bags rows 64:128 are zeroed on ACT/DVE.  MLP
pipelined in 4 N-chunks of 512 (order 2,3,0,1 so score tile 0's B-half
inputs are ready first), producing hT chunks [128, 512] fp16 with rows
64:128 zeroed so all score matmuls engage K=128.

Score tile i (128 T-cols x 2048 N):
  PE:  psB <- chunks 2,3 (+1 zero-filler pass), psA <- chunks 0,1
       (psA/psB are separate [128,1024] PSUM pool tiles so each half
       frees as soon as its reader is done)
  ACT: two 512-col copies psB -> scrA fp16 (starts right after chunk 2)
  DVE: one custom MAXTT_REDUCE_ANT: streams in0=psA (fp32 PSUM) +
       in1=scrA (fp16 SBUF), elementwise max, accum-max over the free
       dim -> colmax[:, i].  No tail reductions anywhere.

Host: segment-max over gathered col maxes + final dot + sigmoid.
"""

import sys
import os

for _p in ("/opt/trn_rl_repo", "/root/.axon_site/_ro/pypackages", "/root/.axon_site"):
    if _p not in sys.path and os.path.isdir(_p):
        sys.path.append(_p)

import numpy as np

from concourse import bass, bacc, tile, mybir
from concourse.bass_utils import run_bass_kernel_spmd

# ---- register the custom DVE op (documented extension point) --------------
from concourse import dve_ops as _dvo
from concourse.dve_spec import Spec as _Spec, Src0 as _Src0, Src1 as _Src1, maxx as _maxx

if "MAXTT_REDUCE_ANT" not in _dvo._SUB_OPCODE_FOR_NAME:
    _MAXTT = _dvo.DveOp(
        "MAXTT_REDUCE_ANT",
        _Spec(body=_maxx(_Src0, _Src1), accum=_maxx),
        subdim=False,
        uops_sha={"v3": "e8861e626b8ad62a", "v4": "7f8046c2b2ccaaf7"},
    )
    _dvo.OPS.append(_MAXTT)
    _dvo.CUSTOM_DVE_SPECS[_MAXTT.name] = _MAXTT.spec
    _dvo._SUB_OPCODE_FOR_NAME[_MAXTT.name] = max(_dvo._SUB_OPCODE_FOR_NAME.values()) + 1
else:
    _MAXTT = next(op for op in _dvo.OPS if op.name == "MAXTT_REDUCE_ANT")

N = 2048
D = 512
T = 100000
R = 100
NCORES = 8
TPC = 12544
NT = TPC // 128  # 98

F32 = mybir.dt.float32
F16 = mybir.dt.float16

KFILL = int(os.environ.get("K_FILL", "1"))      # zero filler passes per tile
NWARM = int(os.environ.get("K_WARM", "4"))     # PE warmup matmuls on zeros


def _build_program():
    nc = bacc.Bacc("TRN2", target_bir_lowering=False, debug=False, num_devices=NCORES)

    xT_d = nc.dram_tensor("xT", [128, 16, 512], F16, kind="ExternalInput")
    w1_d = nc.dram_tensor("w1", [128, 4, 256], F16, kind="ExternalInput")
    w2_d = nc.dram_tensor("w2", [128, 2, 128], F16, kind="ExternalInput")
    w3_d = nc.dram_tensor("w3", [128, 64], F16, kind="ExternalInput")
    bcat_d = nc.dram_tensor("bcat", [128, 4], F32, kind="ExternalInput")
    bags_d = nc.dram_tensor("bags", [64, TPC], F16, kind="ExternalInput")
    out_d = nc.dram_tensor("colmax_out", [128, NT], F32, kind="ExternalOutput")

    relu = mybir.ActivationFunctionType.Relu
    copyf = mybir.ActivationFunctionType.Copy
    amax = mybir.AluOpType.max
    aadd = mybir.AluOpType.add

    with tile.TileContext(nc) as tc:
        with (
            tc.tile_pool(name="const", bufs=1) as cpool,
            tc.tile_pool(name="psA", bufs=2, space="PSUM") as apool,
            tc.tile_pool(name="psB", bufs=2, space="PSUM") as bpool,
        ):
            # ---- zero tiles (memset first: no deps) ----
            zbags_sb = cpool.tile([128, 128], F16, tag="zbags")
            nc.vector.memset(zbags_sb[:, :], 0.0)
            zrhs_sb = cpool.tile([128, 512], F16, tag="zrhs")
            nc.vector.memset(zrhs_sb[:, :], 0.0)
            hT_sb = [
                cpool.tile([128, 512], F16, tag=f"hT{j}", name=f"hT{j}")
                for j in range(4)
            ]
            for j in range(4):
                nc.vector.memset(hT_sb[j][64:128, :], 0.0)

            # ---- DMA loads (multi-queue) ----
            # gpsimd queue: small weights first
            w1_sb = cpool.tile([128, 4, 256], F16, tag="w1p")
            nc.gpsimd.dma_start(w1_sb[:, :, :], w1_d[:, :, :])
            bcat_sb = cpool.tile([128, 4], F32, tag="bcat")
            nc.gpsimd.dma_start(bcat_sb[:, :], bcat_d[:, :])
            b1_sb = [bcat_sb[:, 0:1], bcat_sb[:, 1:2]]
            b2_sb = bcat_sb[:, 2:3]
            b3_sb = bcat_sb[0:64, 3:4]
            w2p_sb = cpool.tile([128, 2, 128], F16, tag="w2p")
            nc.gpsimd.dma_start(w2p_sb[:, :, :], w2_d[:, :, :])
            w2_sb = [w2p_sb[:, 0, :], w2p_sb[:, 1, :]]
            w3_sb = cpool.tile([128, 64], F16, tag="w3")
            nc.gpsimd.dma_start(w3_sb[:], w3_d[:, :])
            # sync queue: xT as 4 strided column-chunk transfers; chunk c
            # covers cols 512c:512(c+1) of ALL k-slices, so L1 chunk j
            # unblocks after one transfer
            # chunk-major xT: [128, 4c+k, 512]; each chunk DMA contiguous
            xT_sb = cpool.tile([128, 16, 512], F16, tag="xTp")
            for c in (2, 3, 0, 1):
                nc.sync.dma_start(
                    xT_sb[:, 4 * c : 4 * (c + 1), :],
                    xT_d[:, 4 * c : 4 * (c + 1), :],
                )
            # bags: real rows on gpsimd queue after the small weights;
            # zero rows 64:128 via idle ACT (memzero) + DVE (memset) early
            bags_sb = cpool.tile([128, TPC], F16, tag="bags")
            nc.scalar.memzero(bags_sb[64:128, 0 : TPC // 4])
            nc.scalar.memzero(bags_sb[64:128, TPC // 4 : TPC // 2])
            nc.vector.memset(bags_sb[64:128, TPC // 2 : 3 * TPC // 4], 0.0)
            nc.vector.memset(bags_sb[64:128, 3 * TPC // 4 : TPC], 0.0)
            BCH = TPC // 8
            for c in range(8):
                nc.gpsimd.dma_start(
                    bags_sb[0:64, BCH * c : BCH * (c + 1)],
                    bags_d[:, BCH * c : BCH * (c + 1)],
                )

            g1_sb = [
                cpool.tile([128, N], F16, tag=f"g1{m}", name=f"g1s{m}")
                for m in range(2)
            ]
            g2_sb = cpool.tile([128, N], F16, tag="g2")
            colmax_sb = cpool.tile([128, NT], F32, tag="colmax")
            scrA = [
                cpool.tile([128, 1024], F16, tag=f"scrA{r}", name=f"scrA{r}")
                for r in range(4)
            ]
            trash = [
                cpool.tile([128, 1024], F16, tag=f"trash{r}", name=f"trash{r}")
                for r in range(1)
            ]

            # ---- PE warmup on zeros (during DMA wait) ----
            for w in range(NWARM):
                pw = apool.tile([128, 1024], F32, tag="psA", name=f"warm{w}")
                nc.tensor.matmul(pw[:, 0:512], zbags_sb[:, :], zrhs_sb[:, :],
                                 start=True, stop=True)

            # ---- MLP, pipelined in 4 N-chunks of 512 ----
            # Chunk order 2,3,0,1: score tile 0 consumes hT2/hT3 first,
            # so it can start after just two MLP chunks.
            for j in (2, 3, 0, 1):
                psa = apool.tile([128, 1024], F32, tag="psA", name=f"psmlpa{j}")
                psb = bpool.tile([128, 1024], F32, tag="psB", name=f"psmlpb{j}")
                sl = slice(512 * j, 512 * (j + 1))
                # L1 -> [256, 512] two m-halves into psa
                for m in range(2):
                    for k in range(4):
                        nc.tensor.matmul(
                            psa[:, 512 * m : 512 * (m + 1)],
                            w1_sb[:, k, 128 * m : 128 * (m + 1)],
                            xT_sb[:, 4 * j + k, :],
                            start=(k == 0),
                            stop=(k == 3),
                        )
                nc.scalar.activation(g1_sb[0][:, sl], psa[:, 0:512], relu,
                                     bias=b1_sb[0])
                nc.vector.tensor_scalar(
                    out=g1_sb[1][:, sl], in0=psa[:, 512:1024],
                    scalar1=b1_sb[1], scalar2=0.0, op0=aadd, op1=amax,
                )
                # L2 -> [128, 512] into psb[:, 0:512]
                for k in range(2):
                    nc.tensor.matmul(
                        psb[:, 0:512], w2_sb[k], g1_sb[k][:, sl],
                        start=(k == 0), stop=(k == 1),
                    )
                nc.vector.tensor_scalar(
                    out=g2_sb[:, sl], in0=psb[:, 0:512],
                    scalar1=b2_sb, scalar2=0.0, op0=aadd, op1=amax,
                )
                # L3 -> [64, 512] into psb[0:64, 512:1024]
                nc.tensor.matmul(
                    psb[0:64, 512:1024], w3_sb[:, :], g2_sb[:, sl],
                    start=True, stop=True,
                )
                nc.scalar.activation(
                    hT_sb[j][0:64, :], psb[0:64, 512:1024], relu, bias=b3_sb
                )

            # ---- score loop ----
            for i in range(NT):
                lhsT = bags_sb[:, 128 * i : 128 * (i + 1)]
                psb = bpool.tile([128, 1024], F32, tag="psB", name=f"pssb{i}")
                psa = apool.tile([128, 1024], F32, tag="psA", name=f"pssa{i}")
                # B half: chunk 2 (clean, so ACT copy 1 starts earliest),
                # then chunk 3 with the zero-filler passes in its group
                nc.tensor.matmul(psb[:, 0:512], lhsT, hT_sb[2][:, :],
                                 start=True, stop=True)
                nc.tensor.matmul(psb[:, 512:1024], lhsT, hT_sb[3][:, :],
                                 start=True, stop=(KFILL == 0))
                for _ in range(KFILL):
                    nc.tensor.matmul(psb[:, 512:1024], zbags_sb[:, :], hT_sb[3][:, :],
                                     start=False, stop=True)
                # A half: chunks 0, 1
                nc.tensor.matmul(psa[:, 0:512], lhsT, hT_sb[0][:, :],
                                 start=True, stop=True)
                nc.tensor.matmul(psa[:, 512:1024], lhsT, hT_sb[1][:, :],
                                 start=True, stop=True)
                # ACT: two 512-col copies so the first starts right after chunk 2
                sA = scrA[i % 4]
                nc.scalar.activation(sA[:, 0:512], psb[:, 0:512], copyf)
                nc.scalar.activation(sA[:, 512:1024], psb[:, 512:1024], copyf)
                # DVE: drain psa + fold scrA, reduce to colmax column
                nc.vector._custom_dve(
                    _MAXTT,
                    out=trash[0][:, :],
                    in0=psa[:, :],
                    in1=sA[:, :],
                    accum_out=colmax_sb[:, i : i + 1],
                )

            nc.sync.dma_start(out_d[:, :], colmax_sb[:])

    nc.compile()
    return nc


_CACHED = {}


def _get_program():
    if "nc" not in _CACHED:
        _CACHED["nc"] = _build_program()
    return _CACHED["nc"]


def _run_device(in_maps, trace=False):
    nc = _get_program()
    try:
        return run_bass_kernel_spmd(nc, in_maps, list(range(NCORES)), trace=trace)
    except ModuleNotFoundError:
        if not trace:
            raise
        return run_bass_kernel_spmd(nc, in_maps, list(range(NCORES)), trace=False)


def _prep_inputs(x, tr_bags, W1, b1, W2, b2, W3, b3):
    xT = np.ascontiguousarray(np.asarray(x, np.float32)[0].T)  # [512, 2048]
    xTp = (xT.astype(np.float16).reshape(4, 128, 4, 512)
           .transpose(1, 2, 0, 3).reshape(128, 16, 512))
    w1p = (np.asarray(W1, np.float32).astype(np.float16)
           .reshape(4, 128, 256).transpose(1, 0, 2))
    w2p = (np.asarray(W2, np.float32).astype(np.float16)
           .reshape(2, 128, 128).transpose(1, 0, 2))
    bcat = np.zeros((128, 4), np.float32)
    bcat[:, 0] = np.asarray(b1, np.float32)[0:128]
    bcat[:, 1] = np.asarray(b1, np.float32)[128:256]
    bcat[:, 2] = np.asarray(b2, np.float32)
    bcat[0:64, 3] = np.asarray(b3, np.float32)
    bags = np.asarray(tr_bags, np.float32)
    bags_pad = np.zeros((64, NCORES * TPC), np.float32)
    bags_pad[:, :T] = bags
    base = {
        "xT": np.ascontiguousarray(xTp),
        "w1": np.ascontiguousarray(w1p),
        "w2": np.ascontiguousarray(w2p),
        "w3": np.ascontiguousarray(np.asarray(W3, np.float32).astype(np.float16)),
        "bcat": bcat,
    }
    in_maps = []
    for c in range(NCORES):
        m = dict(base)
        m["bags"] = np.ascontiguousarray(
            bags_pad[:, c * TPC : (c + 1) * TPC].astype(np.float16)
        )
        in_maps.append(m)
    return in_maps


def _finish_host(colmax, tr_mask, W4, b4):
    tm = np.asarray(tr_mask)
    boundaries = np.searchsorted(tm, np.arange(R + 1))
    ref_max = np.full(R, -np.inf, np.float32)
    nonempty = boundaries[1:] > boundaries[:-1]
    if nonempty.any():
        starts = boundaries[:-1][nonempty]
        ref_max[nonempty] = np.maximum.reduceat(colmax, starts)[: nonempty.sum()]
    z = ref_max.astype(np.float32) @ np.asarray(W4, np.float32) + np.asarray(
        b4, np.float32
    )
    y_prob = (1.0 / (1.0 + np.exp(-z.astype(np.float64)))).astype(np.float32).squeeze()
    y_hat = np.float32(1.0) if y_prob >= 0.5 else np.float32(0.0)
    return np.asarray(y_prob, np.float32), np.asarray(y_hat, np.float32)


def kernel(x, tr_bags, tr_mask, W1, b1, W2, b2, W3, b3, W4, b4, _trace=False):
    in_maps = _prep_inputs(x, tr_bags, W1, b1, W2, b2, W3, b3)
    res = _run_device(in_maps, trace=_trace)
    colmax_parts = []
    for c in range(NCORES):
        cm = res.results[c]["colmax_out"]  # [128, NT]
        colmax_parts.append(np.asarray(cm).T.reshape(-1))
    colmax = np.concatenate(colmax_parts)[:T]
    out = _finish_host(colmax, tr_mask, W4, b4)
    if _trace:
        return out, res
    return out
